# revision 1
# baseline (speedup 1.0000x reference)
"""GRUCell Trainium2 kernel: B=8192, input=hidden=2048, fp32 I/O.

Strategy: data-parallel over batch (1024 rows/core on 8 cores).
Host pre-transposes activations and packs weights so every DMA is
contiguous per partition. Matmuls run in fp16 (fp32 PSUM accumulate):
out[f, b] = sum_k W[f, k] * act[b, k], stationary = 128x128 weight tile,
moving = 512 batch columns. The r/z gates accumulate x@W_i.T and h@W_h.T
into the SAME psum bank (32 k-steps), so the gate pre-activation comes
out of PSUM ready for one ScalarE sigmoid (bias folded in). The n gate
keeps its two halves separate (r multiplies only the h half).
"""

import numpy as np

B = 8192
H = 2048  # hidden == input size
NCORES = 8
BS = B // NCORES  # 1024 batch rows per core
P = 128
KB = H // P   # 16 contraction blocks
FT = H // P   # 16 feature tiles
NF = 512      # psum free width (one bank of fp32)
NB = BS // NF  # 2 batch halves
NW = 6 * KB   # 96 stationary tiles per feature tile

_CACHE = {}


def _build_bass():
    import concourse.bacc as bacc
    import concourse.mybir as mybir
    import concourse.tile as tile

    f16 = mybir.dt.float16
    f32 = mybir.dt.float32
    AF = mybir.ActivationFunctionType
    OP = mybir.AluOpType

    nc = bacc.Bacc(trn_type="TRN2")

    xT = nc.declare_dram_parameter("xT", [P, KB, BS], f16, isOutput=False)
    hT = nc.declare_dram_parameter("hT", [P, KB, BS], f16, isOutput=False)
    wpk = nc.declare_dram_parameter("wpk", [FT, P, NW, P], f16, isOutput=False)
    bpk = nc.declare_dram_parameter("bpk", [P, 4, FT], f32, isOutput=False)
    outT = nc.declare_dram_parameter("outT", [H, BS], f32, isOutput=True)

    with tile.TileContext(nc) as tc:
        with (
            tc.tile_pool(name="res", bufs=1) as res,
            tc.tile_pool(name="wts", bufs=2) as wts,
            tc.tile_pool(name="ew", bufs=2) as ew,
            tc.tile_pool(name="ps", bufs=2, space="PSUM") as ps,
        ):
            xsb = res.tile([P, KB, BS], f16, tag="xsb", bufs=1)
            hsb = res.tile([P, KB, BS], f16, tag="hsb", bufs=1)
            bsb = res.tile([P, 4, FT], f32, tag="bsb", bufs=1)
            nc.sync.dma_start(xsb[:], xT[:])
            nc.sync.dma_start(hsb[:], hT[:])
            nc.sync.dma_start(bsb[:], bpk[:])

            # Priming: the ISA leaves room for very few sync-wait commands
            # per compute instruction, so let each engine observe every DMA
            # semaphore it will depend on once, up front. The Sigmoid also
            # absorbs the one-time ACT table load (sigmoid_and_others covers
            # both Sigmoid and Tanh).
            warm = res.tile([P, 1], f32, tag="warm", bufs=1)
            nc.scalar.activation(warm[:], bsb[:, 0, 0:1], AF.Sigmoid)
            warm2 = res.tile([P, 1], f32, tag="warm2", bufs=1)
            nc.vector.tensor_copy(warm2[:], bsb[:, 1, 0:1])
            warm3 = res.tile([P, 1], f16, tag="warm3", bufs=1)
            nc.vector.tensor_copy(warm3[:], hsb[:, 0, 0:1])

            for ft in range(FT):
                wt = wts.tile([P, NW, P], f16, tag="wt", bufs=2)
                nc.sync.dma_start(wt[:], wpk[ft])
                for bh in range(NB):
                    ps_r = ps.tile([P, NF], f32, tag="ps_r", bufs=2)
                    ps_z = ps.tile([P, NF], f32, tag="ps_z", bufs=2)
                    ps_ni = ps.tile([P, NF], f32, tag="ps_ni", bufs=2)
                    ps_nh = ps.tile([P, NF], f32, tag="ps_nh", bufs=2)

                    bcol = slice(bh * NF, (bh + 1) * NF)
                    # gate r: x@W_ir.T + h@W_hr.T accumulated in one bank
                    for g, (dst, rhs) in enumerate(
                        [
                            (ps_r, xsb), (ps_r, hsb),
                            (ps_z, xsb), (ps_z, hsb),
                            (ps_ni, xsb), (ps_nh, hsb),
                        ]
                    ):
                        first = g % 2 == 0 or g >= 4
                        last = g % 2 == 1 or g >= 4
                        for kb in range(KB):
                            nc.tensor.matmul(
                                dst[:],
                                wt[:, g * KB + kb, :],
                                rhs[:, kb, bcol],
                                start=(first and kb == 0),
                                stop=(last and kb == KB - 1),
                            )

                    r = ew.tile([P, NF], f32, tag="r", bufs=2)
                    z = ew.tile([P, NF], f32, tag="z", bufs=2)
                    t = ew.tile([P, NF], f32, tag="t", bufs=2)
                    s = ew.tile([P, NF], f32, tag="s", bufs=2)
                    n = ew.tile([P, NF], f32, tag="n", bufs=2)
                    d = ew.tile([P, NF], f32, tag="d", bufs=2)
                    o = ew.tile([P, NF], f32, tag="o", bufs=3)

                    nc.scalar.activation(
                        r[:], ps_r[:], AF.Sigmoid, bias=bsb[:, 0, ft : ft + 1]
                    )
                    nc.scalar.activation(
                        z[:], ps_z[:], AF.Sigmoid, bias=bsb[:, 1, ft : ft + 1]
                    )
                    # u = nh + b_hn on ScalarE (Copy w/ bias) so the DVE mult
                    # below has both operands ACT-produced -> a single
                    # cross-engine wait, fitting the crowded 2-src format.
                    u = ew.tile([P, NF], f32, tag="u", bufs=2)
                    nc.scalar.activation(
                        u[:], ps_nh[:], AF.Identity, bias=bsb[:, 3, ft : ft + 1]
                    )
                    nc.vector.tensor_mul(t[:], u[:], r[:])
                    nc.vector.tensor_add(s[:], ps_ni[:], t[:])
                    nc.scalar.activation(
                        n[:], s[:], AF.Tanh, bias=bsb[:, 2, ft : ft + 1]
                    )
                    # h_new = n + z*(h - n)
                    nc.vector.tensor_sub(d[:], hsb[:, ft, bcol], n[:])
                    nc.vector.tensor_mul(d[:], z[:], d[:])
                    nc.vector.tensor_add(o[:], n[:], d[:])
                    nc.sync.dma_start(
                        outT[ft * P : (ft + 1) * P, bcol], o[:]
                    )
    nc.compile()
    return nc


def _prep_inputs(inputs):
    x = inputs["x"]
    h = inputs["h"]
    # [p, kb, b_global]: element = x[b, kb*128+p]
    xT = np.ascontiguousarray(
        x.T.astype(np.float16).reshape(KB, P, B).transpose(1, 0, 2)
    )
    hT = np.ascontiguousarray(
        h.T.astype(np.float16).reshape(KB, P, B).transpose(1, 0, 2)
    )

    wpk = np.empty([FT, P, NW, P], np.float16)
    for g, key in enumerate(["W_ir", "W_hr", "W_iz", "W_hz", "W_in", "W_hn"]):
        WT = inputs[key].T.astype(np.float16)  # [k, f]
        t = WT.reshape(KB, P, FT, P)  # [kb, k_in, ft, f_in]
        wpk[:, :, g * KB : (g + 1) * KB, :] = t.transpose(2, 1, 0, 3)

    b_r = inputs["b_ir"] + inputs["b_hr"]
    b_z = inputs["b_iz"] + inputs["b_hz"]
    bpk = np.stack([b_r, b_z, inputs["b_in"], inputs["b_hn"]]).astype(np.float32)
    # [4, 2048] -> [p, 4, ft]: element = bias_g[ft*128+p]
    bpk = np.ascontiguousarray(bpk.reshape(4, FT, P).transpose(2, 0, 1))

    in_maps = []
    for c in range(NCORES):
        cols = slice(c * BS, (c + 1) * BS)
        in_maps.append(
            {
                "xT": np.ascontiguousarray(xT[:, :, cols]),
                "hT": np.ascontiguousarray(hT[:, :, cols]),
                "wpk": wpk,
                "bpk": bpk,
            }
        )
    return in_maps


def kernel(**inputs):
    from concourse.bass_utils import run_bass_kernel_spmd

    if "nc" not in _CACHE:
        _CACHE["nc"] = _build_bass()
    nc = _CACHE["nc"]
    in_maps = _prep_inputs(inputs)
    res = run_bass_kernel_spmd(nc, in_maps, list(range(NCORES))).results
    outT = np.concatenate([res[c]["outT"] for c in range(NCORES)], axis=1)
    return np.ascontiguousarray(outT.T).astype(np.float32)



# revision 4
# speedup vs baseline: 105.1624x; 105.1624x over previous
"""GRUCell Trainium2 kernel: B=8192, input=hidden=2048, fp32 I/O.

Data-parallel over batch (1024 rows/core on 8 cores). Matmuls in fp16
(fp32 PSUM): out[f, b] = sum_k W[f, k] * act[b, k].

The batch-half loop is INSIDE the stationary-weight
loop — each 128x128 stationary tile is loaded once and used for two
N=512 matmuls (both batch halves), halving LDWEIGHTS count. All 8 PSUM
banks form the live accumulation set (4 gate groups x 2 batch halves);
the per-ft epilogue reads banks in group order so the next ft's matmul
stream overlaps it.
"""

import numpy as np

B = 8192
H = 2048  # hidden == input size
NCORES = 8
BS = B // NCORES  # 1024 batch rows per core
P = 128
KB = H // P   # 16 contraction blocks
FT = H // P   # 16 feature tiles
NF = 512      # psum free width (one bank of fp32)
NB = BS // NF  # 2 batch halves
NW = 6 * KB   # 96 stationary tiles per feature tile

_CACHE = {}


def _build_bass(reps=1):
    import concourse.bacc as bacc
    import concourse.mybir as mybir
    import concourse.tile as tile

    f16 = mybir.dt.float16
    f32 = mybir.dt.float32
    AF = mybir.ActivationFunctionType

    nc = bacc.Bacc(trn_type="TRN2")

    xT = nc.declare_dram_parameter("xT", [P, KB, BS], f16, isOutput=False)
    hT = nc.declare_dram_parameter("hT", [P, KB, BS], f16, isOutput=False)
    wpk = nc.declare_dram_parameter("wpk", [FT, P, NW, P], f16, isOutput=False)
    bpk = nc.declare_dram_parameter("bpk", [P, 4, FT], f32, isOutput=False)
    outT = nc.declare_dram_parameter("outT", [H, BS], f32, isOutput=True)

    with tile.TileContext(nc) as tc:
        with (
            tc.tile_pool(name="res", bufs=1) as res,
            tc.tile_pool(name="wts", bufs=2) as wts,
            tc.tile_pool(name="ew", bufs=3) as ew,
            tc.tile_pool(name="ps", bufs=1, space="PSUM") as ps,
        ):
            xsb = res.tile([P, KB, BS], f16, tag="xsb", bufs=1)
            hsb = res.tile([P, KB, BS], f16, tag="hsb", bufs=1)
            bsb = res.tile([P, 4, FT], f32, tag="bsb", bufs=1)
            nc.sync.dma_start(xsb[:], xT[:])
            nc.sync.dma_start(hsb[:], hT[:])
            nc.sync.dma_start(bsb[:], bpk[:])

            # Prime engines: let each engine observe the activation-DMA
            # semaphores once, and absorb the one-time ACT table load.
            warm = res.tile([P, 1], f32, tag="warm", bufs=1)
            nc.scalar.activation(warm[:], bsb[:, 0, 0:1], AF.Sigmoid)
            warm2 = res.tile([P, 1], f32, tag="warm2", bufs=1)
            nc.vector.tensor_copy(warm2[:], bsb[:, 1, 0:1])
            warm3 = res.tile([P, 1], f16, tag="warm3", bufs=1)
            nc.vector.tensor_copy(warm3[:], hsb[:, 0, 0:1])

            for rep in range(reps):
              for ft in range(FT):
                wt = wts.tile([P, NW, P], f16, tag="wt", bufs=2)
                nc.sync.dma_start(wt[:], wpk[ft])

                pr = [ps.tile([P, NF], f32, name=f"ps_r{b}", tag=f"ps_r{b}", bufs=1) for b in range(NB)]
                pz = [ps.tile([P, NF], f32, name=f"ps_z{b}", tag=f"ps_z{b}", bufs=1) for b in range(NB)]
                pni = [ps.tile([P, NF], f32, name=f"ps_ni{b}", tag=f"ps_ni{b}", bufs=1) for b in range(NB)]
                pnh = [ps.tile([P, NF], f32, name=f"ps_nh{b}", tag=f"ps_nh{b}", bufs=1) for b in range(NB)]

                bcols = [slice(b * NF, (b + 1) * NF) for b in range(NB)]
                for g, (dst, rhs) in enumerate(
                    [
                        (pr, xsb), (pr, hsb),
                        (pz, xsb), (pz, hsb),
                        (pni, xsb), (pnh, hsb),
                    ]
                ):
                    first = g % 2 == 0 or g >= 4
                    last = g % 2 == 1 or g >= 4
                    for kb in range(KB):
                        w = wt[:, g * KB + kb, :]
                        for b in range(NB):
                            nc.tensor.matmul(
                                dst[b][:],
                                w,
                                rhs[:, kb, bcols[b]],
                                start=(first and kb == 0),
                                stop=(last and kb == KB - 1),
                            )

                for b in range(NB):
                    bcol = bcols[b]
                    r = ew.tile([P, NF], f32, tag=f"r{b}", bufs=2)
                    z = ew.tile([P, NF], f32, tag=f"z{b}", bufs=2)
                    u = ew.tile([P, NF], f32, tag=f"u{b}", bufs=2)
                    t = ew.tile([P, NF], f32, tag=f"t{b}", bufs=2)
                    s = ew.tile([P, NF], f32, tag=f"s{b}", bufs=2)
                    n = ew.tile([P, NF], f32, tag=f"n{b}", bufs=2)
                    d = ew.tile([P, NF], f32, tag=f"d{b}", bufs=2)
                    o = ew.tile([P, NF], f32, tag=f"o{b}", bufs=3)

                    nc.scalar.activation(
                        r[:], pr[b][:], AF.Sigmoid, bias=bsb[:, 0, ft : ft + 1]
                    )
                    nc.scalar.activation(
                        z[:], pz[b][:], AF.Sigmoid, bias=bsb[:, 1, ft : ft + 1]
                    )
                    # u = nh + b_hn on ScalarE so the DVE mult below has both
                    # operands ACT-produced (single cross-engine wait).
                    nc.scalar.activation(
                        u[:], pnh[b][:], AF.Identity, bias=bsb[:, 3, ft : ft + 1]
                    )
                    nc.vector.tensor_mul(t[:], u[:], r[:])
                    nc.vector.tensor_add(s[:], pni[b][:], t[:])
                    nc.scalar.activation(
                        n[:], s[:], AF.Tanh, bias=bsb[:, 2, ft : ft + 1]
                    )
                    # h_new = n + z*(h - n)
                    nc.vector.tensor_sub(d[:], hsb[:, ft, bcol], n[:])
                    nc.vector.tensor_mul(d[:], z[:], d[:])
                    nc.vector.tensor_add(o[:], n[:], d[:])
                    nc.sync.dma_start(
                        outT[ft * P : (ft + 1) * P, bcol], o[:]
                    )
    nc.compile()
    return nc


def _prep_inputs(inputs):
    x = inputs["x"]
    h = inputs["h"]
    # [p, kb, b_global]: element = x[b, kb*128+p]
    xT = np.ascontiguousarray(
        x.T.astype(np.float16).reshape(KB, P, B).transpose(1, 0, 2)
    )
    hT = np.ascontiguousarray(
        h.T.astype(np.float16).reshape(KB, P, B).transpose(1, 0, 2)
    )

    wpk = np.empty([FT, P, NW, P], np.float16)
    for g, key in enumerate(["W_ir", "W_hr", "W_iz", "W_hz", "W_in", "W_hn"]):
        WT = inputs[key].T.astype(np.float16)  # [k, f]
        t = WT.reshape(KB, P, FT, P)  # [kb, k_in, ft, f_in]
        wpk[:, :, g * KB : (g + 1) * KB, :] = t.transpose(2, 1, 0, 3)

    b_r = inputs["b_ir"] + inputs["b_hr"]
    b_z = inputs["b_iz"] + inputs["b_hz"]
    bpk = np.stack([b_r, b_z, inputs["b_in"], inputs["b_hn"]]).astype(np.float32)
    # [4, 2048] -> [p, 4, ft]: element = bias_g[ft*128+p]
    bpk = np.ascontiguousarray(bpk.reshape(4, FT, P).transpose(2, 0, 1))

    in_maps = []
    for c in range(NCORES):
        cols = slice(c * BS, (c + 1) * BS)
        in_maps.append(
            {
                "xT": np.ascontiguousarray(xT[:, :, cols]),
                "hT": np.ascontiguousarray(hT[:, :, cols]),
                "wpk": wpk,
                "bpk": bpk,
            }
        )
    return in_maps


def kernel(**inputs):
    from concourse.bass_utils import run_bass_kernel_spmd

    if "nc" not in _CACHE:
        _CACHE["nc"] = _build_bass()
    nc = _CACHE["nc"]
    in_maps = _prep_inputs(inputs)
    res = run_bass_kernel_spmd(nc, in_maps, list(range(NCORES))).results
    outT = np.concatenate([res[c]["outT"] for c in range(NCORES)], axis=1)
    return np.ascontiguousarray(outT.T).astype(np.float32)
